# revision 1
# baseline (speedup 1.0000x reference)
"""AxialAttention2D kernel for 8 TRN2 NeuronCores.

Sharding: data-parallel over B (B == 8 == n_cores). Each core processes one
full [C, H, W] image: both the height pass (attend along W for each row h)
and the width pass (attend along H for each column w), accumulating
(xh + xw) / 2 into an SBUF-resident fp32 accumulator. No collectives.

Inner structure: super-groups of SG=4 items (2 matmul sub-groups of 2),
scores row-tiled across 4 PSUM banks (concurrent row-tiled matmuls must
write different banks - HW constraint), one 2048-wide exp per super-group,
softmax row-sums via per-head N=512 ones-matmuls (col-tiled), projection
with 0.5/bias folded into host-side weights.

Self-contained: shapes are hardcoded (B=8, C=128, H=W=128, heads=4).
"""

import numpy as np
from contextlib import ExitStack

C = 128          # channels (= SBUF partitions)
L = 128          # attention sequence length (H or W)
HW = L * L       # flattened spatial size
HEADS = 4
HD = C // HEADS  # 32
SCALE = HD ** -0.5
SG = 4           # items per super-group
S_ITEMS = 40     # phase-0 H-items (overlap input DMA); multiple of SG
NCHUNK = 16
CHW = HW // NCHUNK  # 1024 columns per input chunk

_cache = {}

W_NAMES = ("wqT_h", "wkT_h", "wvT_h", "wpT_h", "wqT_w", "wkT_w", "wvT_w", "wpT_w")


def _build_nc():
    import concourse.bacc as bacc
    import concourse.tile as tile
    from concourse import mybir

    f32 = mybir.dt.float32
    bf16 = mybir.dt.bfloat16
    Exp = mybir.ActivationFunctionType.Exp
    Ident = mybir.ActivationFunctionType.Identity

    nc = bacc.Bacc(None, name="axial_attn")

    x_d = nc.dram_tensor("x", [C, HW], f32, kind="ExternalInput")
    w_d = {n: nc.dram_tensor(n, [C, C], bf16, kind="ExternalInput") for n in W_NAMES}
    bias_d = nc.dram_tensor("bias", [C, 1], f32, kind="ExternalInput")
    out_d = nc.dram_tensor("out", [C, HW], f32, kind="ExternalOutput")

    with ExitStack() as ctx:
        tc = ctx.enter_context(tile.TileContext(nc))
        singles = ctx.enter_context(tc.tile_pool(name="singles", bufs=1))
        big = ctx.enter_context(tc.tile_pool(name="big", bufs=1))
        work = ctx.enter_context(tc.tile_pool(name="work", bufs=3))
        nrm = ctx.enter_context(tc.tile_pool(name="nrm", bufs=2))
        # PSUM: s(4 banks, bufs=1) + qk(tag-shared q/k, bufs=2 -> 2 banks)
        #       + vap(tag-shared vT/av/rs/p, bufs=2 -> 2 banks) = 8 banks
        ps_s = ctx.enter_context(tc.tile_pool(name="ps_s", bufs=1, space="PSUM"))
        ps_qk = ctx.enter_context(tc.tile_pool(name="ps_qk", bufs=2, space="PSUM"))
        ps_vap = ctx.enter_context(tc.tile_pool(name="ps_vap", bufs=2, space="PSUM"))

        w_sb = {}
        for n in W_NAMES:
            w_sb[n] = singles.tile([C, C], bf16, tag=n, name=n)
            nc.sync.dma_start(out=w_sb[n][:], in_=w_d[n][:])
        bias_sb = singles.tile([C, 1], f32, tag="bias")
        nc.sync.dma_start(out=bias_sb[:], in_=bias_d[:])
        ones_sb = singles.tile([C, HD], bf16, tag="ones")
        nc.vector.memset(ones_sb[:], 1.0)

        x_sb = big.tile([C, HW], f32, tag="x_f32")
        xc = big.tile([C, HW], bf16, tag="x_bf16")
        acc = big.tile([C, HW], f32, tag="acc")

        # Strided views for the width pass: free dims become (w, h)
        xc_v = xc[:].rearrange("c (h w) -> c w h", w=L)
        acc_v = acc[:].rearrange("c (h w) -> c w h", w=L)

        def load_chunk(ci, eng=None):
            sl = slice(ci * CHW, (ci + 1) * CHW)
            nc.sync.dma_start(out=x_sb[:, sl], in_=x_d[:, sl])
            eng = eng or nc.gpsimd
            if eng is nc.scalar:
                eng.copy(out=xc[:, sl], in_=x_sb[:, sl])
            else:
                eng.tensor_copy(out=xc[:, sl], in_=x_sb[:, sl])

        def sgroup(passc, g0, mode):
            """Process items g0..g0+SG-1 of one pass.

            passc: 'h' (items are rows, attend along w) or 'w'.
            mode: 'init' -> acc = proj + bias      (phase-0 H-groups)
                  'w'    -> split add/init         (phase-1 W-groups)
                  'add'  -> acc += proj, DMA out   (phase-2 H-groups)
            """
            wq, wk, wv, wp = (w_sb[f"w{t}T_{passc}"] for t in ("q", "k", "v", "p"))
            SL = SG * L  # 512

            def xs_item(it):
                if passc == "h":
                    return xc[:, (g0 + it) * L:(g0 + it + 1) * L]
                return xc_v[:, g0 + it, :]

            # QKV.  q,k each as one N=512 matmul over all 4 items; vT per item.
            if passc == "h":
                rhs_qk = xc[:, g0 * L:(g0 + SG) * L]
            else:
                rhs_qk = xc_v[:, g0:g0 + SG, :]
            q_ps = ps_qk.tile([C, SL], f32, tag="qk", name="q_ps")
            nc.tensor.matmul(q_ps[:], wq[:], rhs_qk, start=True, stop=True)
            k_ps = ps_qk.tile([C, SL], f32, tag="qk", name="k_ps")
            nc.tensor.matmul(k_ps[:], wk[:], rhs_qk, start=True, stop=True)
            va = ps_vap.tile([C, SL], f32, tag="vap", name="vt_ps")
            for it in range(SG):
                nc.tensor.matmul(va[:, it * L:(it + 1) * L], xs_item(it), wv[:],
                                 start=True, stop=True)
            # layout: [q0..q3 | k0..k3 | vT0..vT3] (bf16)
            qkv_sb = work.tile([C, 3 * SL], bf16, tag="qkv_sb")
            nc.vector.tensor_copy(out=qkv_sb[:, 0:SL], in_=q_ps[:])
            nc.vector.tensor_copy(out=qkv_sb[:, SL:2 * SL], in_=k_ps[:])
            # vT copy for all 4 items at offset 1024 (ScalarE for balance)
            nc.scalar.copy(out=qkv_sb[:, 1024:1536], in_=va[:])

            # scores (transposed): sT_h[j, i] = sum_d k[d,j] q[d,i], row-tiled.
            # HW: concurrent row-tiled matmuls need different PSUM banks ->
            # head h -> bank h.  Layout: offset = h*512 + sub*256 + gl*128.
            s_ps = ps_s.tile([C, 2048], f32, tag="s")
            for it in range(SG):
                for h in range(HEADS):
                    off = h * 512 + it * L
                    qoff = it * L
                    koff = SL + it * L
                    nc.tensor.matmul(
                        s_ps[:, off:off + L],
                        qkv_sb[HD * h:HD * h + HD, koff:koff + L],
                        qkv_sb[HD * h:HD * h + HD, qoff:qoff + L],
                        start=True, stop=True, tile_position=(HD * h, 0))

            # exp in two bank-pair halves so s-banks free incrementally and
            # the next group's score matmuls can overlap the second half
            eT = work.tile([C, 2048], bf16, tag="eT")
            nc.scalar.activation(out=eT[:, 0:1024], in_=s_ps[:, 0:1024],
                                 func=Exp, scale=SCALE)
            nc.scalar.activation(out=eT[:, 1024:2048], in_=s_ps[:, 1024:2048],
                                 func=Exp, scale=SCALE)

            # A@V col-tiled per (item, head); rowsums via per-head N=512
            # ones-matmuls.  Column order of both: (it, i).
            av = ps_vap.tile([C, SL], f32, tag="vap", name="av_ps")
            rs = ps_vap.tile([C, SL], f32, tag="vap", name="rs_ps")
            for h in range(HEADS):
                nc.tensor.matmul(rs[HD * h:HD * h + HD, :], ones_sb[:],
                                 eT[:, h * 512:(h + 1) * 512],
                                 start=True, stop=True, tile_position=(0, HD * h))
            for it in range(SG):
                for h in range(HEADS):
                    esl = eT[:, h * 512 + it * L:h * 512 + (it + 1) * L]
                    nc.tensor.matmul(
                        av[HD * h:HD * h + HD, it * L:(it + 1) * L],
                        qkv_sb[:, 1024 + it * L + HD * h:1024 + it * L + HD * h + HD],
                        esl, start=True, stop=True, tile_position=(0, HD * h))

            rr = nrm.tile([C, SL], f32, tag="rr")
            nc.vector.reciprocal_approx_fast(out=rr[:], in_=rs[:])
            on = nrm.tile([C, SL], bf16, tag="on")
            nc.vector.tensor_mul(out=on[:], in0=av[:], in1=rr[:])

            p_ps = ps_vap.tile([C, SL], f32, tag="vap", name="p_ps")
            nc.tensor.matmul(p_ps[:], wp[:], on[:], start=True, stop=True)

            if mode == "init":
                nc.scalar.activation(out=acc[:, g0 * L:(g0 + SG) * L], in_=p_ps[:],
                                     func=Ident, bias=bias_sb[:], scale=1.0)
            elif mode == "w":
                accv = acc_v[:, g0:g0 + SG, :]
                pv = p_ps[:].rearrange("c (g l) -> c g l", g=SG)
                # items 0..S_ITEMS-1 were initialized in phase-0 -> add
                nc.vector.tensor_add(out=accv[:, :, 0:S_ITEMS],
                                     in0=pv[:, :, 0:S_ITEMS],
                                     in1=accv[:, :, 0:S_ITEMS])
                # rest: first write, carries the bias
                nc.scalar.activation(out=accv[:, :, S_ITEMS:L],
                                     in_=pv[:, :, S_ITEMS:L],
                                     func=Ident, bias=bias_sb[:], scale=1.0)
            else:  # "add"
                blk = acc[:, g0 * L:(g0 + SG) * L]
                nc.vector.tensor_add(out=blk, in0=p_ps[:], in1=blk)
                nc.sync.dma_start(out=out_d[:, g0 * L:(g0 + SG) * L], in_=blk)

        # ---- schedule ----
        # phase-0: first S_ITEMS height items, overlapping the input stream
        n_s_chunks = S_ITEMS * L // CHW  # 5
        for ci in range(n_s_chunks):
            load_chunk(ci)
        s_groups = list(range(0, S_ITEMS, SG))
        # late chunks: 5-8 cast on gpsimd right away (idle engine), the rest
        # on vector/scalar interleaved with late phase-0 groups
        for ci in (5, 6, 7, 8):
            load_chunk(ci)
        late = [(9, nc.vector), (10, nc.scalar), (11, nc.vector), (12, nc.scalar),
                (13, nc.vector), (14, nc.scalar), (15, nc.vector)]
        for i, g0 in enumerate(s_groups):
            sgroup("h", g0, "init")
            if i >= 3 and late:
                ci, eng = late.pop(0)
                load_chunk(ci, eng=eng)
        for ci, eng in late:
            load_chunk(ci, eng=eng)
        # phase-1: width pass (needs the full image)
        for g0 in range(0, L, SG):
            sgroup("w", g0, "w")
        # phase-2: remaining height items, streaming output
        s_out_chunks = list(range(n_s_chunks))  # cols 0..S_ITEMS*L final now
        for i, g0 in enumerate(range(S_ITEMS, L, SG)):
            sgroup("h", g0, "add")
            if i < len(s_out_chunks):
                ci = s_out_chunks[i]
                sl = slice(ci * CHW, (ci + 1) * CHW)
                nc.sync.dma_start(out=out_d[:, sl], in_=acc[:, sl])

    nc.finalize()
    return nc


def _get_nc():
    if "nc" not in _cache:
        _cache["nc"] = _build_nc()
    return _cache["nc"]


def _make_in_maps(x, wqkv_h, wproj_h, bproj_h, wqkv_w, wproj_w, bproj_w):
    import ml_dtypes
    bf = ml_dtypes.bfloat16
    x = np.asarray(x, dtype=np.float32)
    B = x.shape[0]

    def wT(w):
        return np.ascontiguousarray(np.asarray(w, np.float32).T)

    common = {
        "wqT_h": wT(wqkv_h[0:C]).astype(bf),
        "wkT_h": wT(wqkv_h[C:2 * C]).astype(bf),
        "wvT_h": wT(wqkv_h[2 * C:3 * C]).astype(bf),
        "wpT_h": (wT(wproj_h) * 0.5).astype(bf),
        "wqT_w": wT(wqkv_w[0:C]).astype(bf),
        "wkT_w": wT(wqkv_w[C:2 * C]).astype(bf),
        "wvT_w": wT(wqkv_w[2 * C:3 * C]).astype(bf),
        "wpT_w": (wT(wproj_w) * 0.5).astype(bf),
        "bias": (0.5 * (np.asarray(bproj_h, np.float32)
                        + np.asarray(bproj_w, np.float32))).reshape(C, 1),
    }
    return [
        {**common, "x": np.ascontiguousarray(x[b].reshape(C, HW))}
        for b in range(B)
    ]


def _run(in_maps, **kw):
    from concourse.bass_utils import run_bass_kernel_spmd
    nc = _get_nc()
    res = run_bass_kernel_spmd(nc, in_maps, core_ids=list(range(len(in_maps))), **kw)
    _cache["last_results"] = res
    return res


def kernel(x, wqkv_h, wproj_h, bproj_h, wqkv_w, wproj_w, bproj_w):
    in_maps = _make_in_maps(x, wqkv_h, wproj_h, bproj_h,
                            wqkv_w, wproj_w, bproj_w)
    res = _run(in_maps)
    out = np.stack([r["out"].reshape(C, L, L) for r in res.results], axis=0)
    return out.astype(np.float32)

